# revision 26
# baseline (speedup 1.0000x reference)
"""Trainium2 Bass kernel for causal ReLU attention (no softmax).

  qkv = x @ W.T + b;  per head: s = (q k^T) * 1/sqrt(64)
  p = relu(causal(s));  y = p @ v

Sharding: 8 cores = 2 batches x 4 head-groups (3 heads each).

Per-core structure (v2):
  - qk-projection computed transposed (features on partitions) so q/k land
    as qT/kT [64, T]; scores use K=128 zero-padded k tiles (K<128 disables
    FWL and is ~4x slower per measurement; padding is free).
  - v-projection natural [T, 192]; pv matmuls are plain M=64 K=128 into
    per-chain [64, 512] PSUM tiles (tile_position pairing measured slower).
  - attention runs as 6 chain-pairs stepped in lockstep; relu+causal-mask
    is one fused op per step covering both chains' [128, 512] score
    blocks; diagonal masking via slices of master/twin triangle tiles
    (DVE scalar_tensor_tensor).  Engine per step chosen by greedy load
    balance (masked steps must use DVE).
  - global software pipeline: SC(i+3), RL(i+2), PV(i) with 3 score
    PSUM tiles (6 banks) + ytp/proj pool (2 banks) = 8 banks.
  - all DMA on the sync ring (scalar-ring DMA measured ~9x slower).
All matmul operands fp16 (fp32 PSUM accumulation). Host does the
shard/transpose/cast prep and the final gather (pure numpy).
"""
import numpy as np

import concourse.bass as bass
import concourse.mybir as mybir
import concourse.tile as tile
from concourse import bacc
from concourse.bass_utils import run_bass_kernel_spmd

F32 = mybir.dt.float32
F16 = mybir.dt.float16

B, T, C = 2, 2048, 768
NH = 12          # total heads
HPC = 3          # heads per core
D = 64
NCORES = 8
CC = 6           # contraction chunks (768 / 128)
TB = 512         # query block
KB = 128         # key block
NTB = T // TB    # 4
NKB = T // KB    # 16
MW = 1664        # master mask width

# cost model (ns) for engine balancing (ACT has measured 1.16x derate)
def _dve_cost(w):
    return (151.0 + w) / 0.96

def _act_cost(w):
    return 1.16 * (352.0 + w) / 1.2


def _build(reps=1, stage=4):
    nc = bacc.Bacc(None, target_bir_lowering=False, debug=False)
    xT = nc.declare_dram_parameter("xT", [C, T], F16, isOutput=False)
    wqk = nc.declare_dram_parameter("wqk", [C, 384], F16, isOutput=False)
    wv = nc.declare_dram_parameter("wv", [C, 192], F16, isOutput=False)
    bias_qk = nc.declare_dram_parameter("bias_qk", [3, 128], F32, isOutput=False)
    scale_qk = nc.declare_dram_parameter("scale_qk", [3, 128], F32, isOutput=False)
    bias_v = nc.declare_dram_parameter("bias_v", [128, 1024], F32, isOutput=False)
    yt_out = nc.declare_dram_parameter("yt", [HPC, D, T], F32, isOutput=True)

    with tile.TileContext(nc) as tc:
        with tc.tile_pool(name="const", bufs=1) as const, \
             tc.tile_pool(name="xr", bufs=12) as xr, \
             tc.tile_pool(name="vt", bufs=16) as vtp, \
             tc.tile_pool(name="pt", bufs=6) as ptp, \
             tc.tile_pool(name="ys", bufs=4) as ysp, \
             tc.tile_pool(name="psmix", bufs=2, space="PSUM") as psmix, \
             tc.tile_pool(name="pssc", bufs=3, space="PSUM") as pssc:

            # ---------------- constants ----------------
            bias_sb = const.tile([128, 3], F32)
            scale_sb = const.tile([128, 3], F32)
            nc.sync.dma_start(out=bias_sb, in_=bias_qk[:, :].rearrange("a p -> p a"))
            nc.sync.dma_start(out=scale_sb, in_=scale_qk[:, :].rearrange("a p -> p a"))
            biasv_sb = const.tile([128, 1024], F32)
            nc.sync.dma_start(out=biasv_sb, in_=bias_v[:, :])
            # master triangle: master[kk, m] = 1 if m >= 512 + kk else 0
            #   masked slice  : master[:, 512 : 512+w]  (diag at op-local col 0)
            #   all-ones slice: master[:, 640 : 640+w]
            master = const.tile([128, MW], F32)
            nc.vector.memset(master, 1.0)
            nc.gpsimd.affine_select(
                out=master, in_=master,
                compare_op=mybir.AluOpType.is_ge, fill=0.0, base=-512,
                pattern=[[1, MW]], channel_multiplier=-1)
            # twin triangles: twin[kk, m] = 1 if (m % 512) >= kk (for steps
            # where BOTH chains are diagonal blocks at the same shift)
            twin = const.tile([128, 2 * TB], F32)
            nc.vector.memset(twin, 1.0)
            for hh in range(2):
                nc.gpsimd.affine_select(
                    out=twin[:, hh * TB:(hh + 1) * TB],
                    in_=twin[:, hh * TB:(hh + 1) * TB],
                    compare_op=mybir.AluOpType.is_ge, fill=0.0, base=0,
                    pattern=[[1, TB]], channel_multiplier=-1)

            # weight chunks
            wqk_sb = [const.tile([128, 384], F16, tag="wqk", bufs=CC, name=f"wqk{c}") for c in range(CC)]
            wv_sb = [const.tile([128, 192], F16, tag="wv", bufs=CC, name=f"wv{c}") for c in range(CC)]
            for c in range(CC):
                nc.sync.dma_start(out=wqk_sb[c], in_=wqk[c * 128:(c + 1) * 128, :])
                nc.sync.dma_start(out=wv_sb[c], in_=wv[c * 128:(c + 1) * 128, :])

            # persistent attention operand tiles (zero-padded k selects the
            # head: contraction over 128 partitions hits zeros for the other)
            qq01 = const.tile([128, T], F16)   # [qT_h0; qT_h1] (scaled)
            kzA = const.tile([128, T], F16)    # [kT_h0; 0]
            kzB = const.tile([128, T], F16)    # [0; kT_h1]
            qq2 = const.tile([128, T], F16)    # [qT_h2 (dma); qT_h2 (act)]
            kz2 = const.tile([128, T], F16)    # [kT_h2; 0]
            nc.vector.memset(kzA[64:128, :], 0.0)
            nc.vector.memset(kzB[0:64, :], 0.0)
            nc.vector.memset(kz2[64:128, :], 0.0)

            def body():
                # stage: 1=dma only, 2=+proj, 3=+scores/relu, 4=full
                # probe stages: 21=xt+qkproj only, 22=xt+vproj only
                # ---------------- load xT (fp16, pre-cast on host) ---------
                xt = [xr.tile([128, T], F16, tag="xt", name=f"xt{c}") for c in range(CC)]
                for c in range(CC):
                    nc.sync.dma_start(out=xt[c], in_=xT[c * 128:(c + 1) * 128, :])

                if stage < 2:
                    return
                # ---------------- qk projection (transposed) ---------------
                # f-tiles: 0 = [q0; q1], 1 = [k0; k1], 2 = [k2; q2]
                # Two query-blocks per [128,1024] PSUM tile; two tiles'
                # accumulation chains interleaved so PSUM drains overlap
                # the next chain's streaming (chained mms into one region
                # otherwise serialize at ~490ns vs 136ns independent).
                Copy = mybir.ActivationFunctionType.Identity
                proj_eng = [0]

                def qk_emit(tiles):
                    # tiles: list of (ft, tbp) -> one [128,1024] psum tile
                    ps_of = []
                    for ft, tbp in tiles:
                        ps = pssc.tile([128, 2 * TB], F32, tag="s",
                                       name=f"pj{ft}_{tbp}")
                        ps_of.append(ps)
                    for c in range(CC):
                        for (ft, tbp), ps in zip(tiles, ps_of):
                            for h2 in range(2):
                                nc.tensor.matmul(
                                    ps[:, h2 * TB:(h2 + 1) * TB],
                                    wqk_sb[c][:, ft * 128:(ft + 1) * 128],
                                    xt[c][:, (tbp + h2) * TB:(tbp + h2 + 1) * TB],
                                    start=(c == 0), stop=(c == CC - 1))
                    for (ft, tbp), ps in zip(tiles, ps_of):
                        ts = slice(tbp * TB, (tbp + 2) * TB)
                        if ft == 0:
                            dsts = [(qq01, slice(0, 128), 0)]
                        elif ft == 1:
                            dsts = [(kzA, slice(0, 64), 1),
                                    (kzB, slice(64, 128), 1)]
                        else:
                            dsts = [(kz2, slice(0, 64), 2),
                                    (qq2, slice(64, 128), 2)]
                        for dst, psl, col in dsts:
                            if proj_eng[0] % 2 == 0:
                                nc.scalar.activation(
                                    dst[psl, ts], ps[psl, :], Copy,
                                    bias=bias_sb[psl, col:col + 1],
                                    scale=scale_sb[psl, col:col + 1])
                            else:
                                nc.vector.tensor_scalar(
                                    out=dst[psl, ts], in0=ps[psl, :],
                                    scalar1=scale_sb[psl, col:col + 1],
                                    scalar2=bias_sb[psl, col:col + 1],
                                    op0=mybir.AluOpType.mult,
                                    op1=mybir.AluOpType.add)
                            proj_eng[0] += 1

                if stage != 22:
                    qk_emit([(0, 0), (0, 2)])
                    qk_emit([(1, 0), (1, 2)])
                    qk_emit([(2, 0), (2, 2)])
                # shift qT_h2 to partitions 0-63 (SBUF->SBUF DMA)
                if stage not in (21, 22):
                    nc.sync.dma_start(out=qq2[0:64, :], in_=qq2[64:128, :])

                # ---------------- v projection (natural layout) ------------
                # One accumulation group per PSUM BANK (start=True clears
                # the whole bank, so groups must not share one): 2 token-
                # blocks per [128,1024] tile at cols 0/512; tile PAIRS
                # interleave for 4 independent chains (drain hiding).
                v2_sb = []
                if stage != 21:
                    for tp_ in range(4):
                        pss = [pssc.tile([128, 2 * TB], F32, tag="s",
                                         name=f"pv{tp_}_{j}") for j in range(2)]
                        for c in range(CC):
                            for j in range(2):
                                for blk in range(2):
                                    tt = tp_ * 4 + j * 2 + blk
                                    nc.tensor.matmul(
                                        pss[j][:, blk * TB:blk * TB + 192],
                                        xt[c][:, tt * 128:(tt + 1) * 128],
                                        wv_sb[c],
                                        start=(c == 0), stop=(c == CC - 1))
                        for j in range(2):
                            vt = vtp.tile([128, 2 * TB], F16, tag="v")
                            for blk in range(2):
                                nc.vector.tensor_add(
                                    vt[:, blk * TB:blk * TB + 192],
                                    pss[j][:, blk * TB:blk * TB + 192],
                                    biasv_sb[:, blk * TB:blk * TB + 192])
                            v2_sb.append(vt)

                if stage < 3 or stage in (21, 22):
                    return
                # ---------------- attention ----------------
                # chain operand lookup: head -> (q tile, zero-padded k tile)
                qk_of = {0: (qq01, kzA), 1: (qq01, kzB), 2: (qq2, kz2)}

                # pairs of (head, qb); first chain on partition half 0,
                # second on half 1.
                pairs = [((0, 0), (1, 0)), ((0, 1), (1, 1)),
                         ((0, 2), (1, 2)), ((0, 3), (1, 3)),
                         ((2, 0), (2, 1)), ((2, 2), (2, 3))]

                # ---- build step descriptors ----
                # entry: dict(head, qb, half, kb, lo(None if full), cb,
                #             start, stop)
                steps = []
                for pi, (ca, cb_) in enumerate(pairs):
                    chains = []
                    for half, (h, qb) in enumerate((ca, cb_)):
                        chains.append(dict(head=h, qb=qb, half=half,
                                           nkb=4 * qb + 4))
                    S = max(c["nkb"] for c in chains)
                    for t in range(S):
                        ents = []
                        for cch in chains:
                            if t >= cch["nkb"]:
                                continue
                            nfull = cch["nkb"] - 4
                            lo = None if t < nfull else (t - nfull) * KB
                            ents.append(dict(head=cch["head"], qb=cch["qb"],
                                             half=cch["half"], kb=t, lo=lo,
                                             start=(t == 0),
                                             stop=(t == cch["nkb"] - 1)))
                        # diag chain (if exactly one) goes in sp cols 0-511
                        if len(ents) == 2 and ents[0]["lo"] is None \
                                and ents[1]["lo"] is not None:
                            ents = [ents[1], ents[0]]
                        for j, e in enumerate(ents):
                            e["cb"] = j * TB
                        masked = any(e["lo"] is not None for e in ents)
                        op_lo = ents[0]["lo"] if ents[0]["lo"] is not None else 0
                        op_hi = TB if len(ents) == 1 else 2 * TB
                        steps.append(dict(pair=pi, t=t, ents=ents,
                                          masked=masked, op_lo=op_lo,
                                          op_hi=op_hi,
                                          last=(t == S - 1)))

                # ---- engine assignment (greedy balance) ----
                eng_t = {"dve": 0.0, "act": 0.0}
                for st in steps:
                    w = st["op_hi"] - st["op_lo"]
                    if st["masked"]:
                        st["eng"] = "dve"
                        eng_t["dve"] += _dve_cost(w)
                    else:
                        if eng_t["act"] + _act_cost(w) <= eng_t["dve"] + _dve_cost(w):
                            st["eng"] = "act"
                            eng_t["act"] += _act_cost(w)
                        else:
                            st["eng"] = "dve"
                            eng_t["dve"] += _dve_cost(w)
                    if st["last"]:  # two [64,512] ys copies follow
                        st["ys_eng"] = []
                        for _ in range(2):
                            if eng_t["act"] <= eng_t["dve"]:
                                st["ys_eng"].append("act")
                                eng_t["act"] += _act_cost(TB)
                            else:
                                st["ys_eng"].append("dve")
                                eng_t["dve"] += _dve_cost(TB)

                # ---- emission helpers ----
                ytp_of = {}

                def emit_sc(st):
                    sp = pssc.tile([128, 2 * TB], F32, tag="s",
                                   name=f"sp{st['pair']}_{st['t']}")
                    st["sp"] = sp
                    for e in st["ents"]:
                        qsb, ksb = qk_of[e["head"]]
                        qb, kb, cb = e["qb"], e["kb"], e["cb"]
                        lo = e["lo"]
                        # both-diag steps: second (full-slot) chain emitted
                        # unrestricted; garbage cols masked-out / unread.
                        if lo is None or (cb == TB):
                            q_ap = qsb[:, qb * TB:(qb + 1) * TB]
                            out_ap = sp[:, cb:cb + TB]
                        else:
                            q_ap = qsb[:, qb * TB + lo:(qb + 1) * TB]
                            out_ap = sp[:, cb + lo:cb + TB]
                        nc.tensor.matmul(
                            out_ap, ksb[:, kb * KB:(kb + 1) * KB], q_ap,
                            start=True, stop=True)

                def emit_rl(st):
                    pt = ptp.tile([128, 2 * TB], F16, tag="p2",
                                  name=f"pt{st['pair']}_{st['t']}")
                    st["pt"] = pt
                    lo, hi = st["op_lo"], st["op_hi"]
                    w = hi - lo
                    if st["masked"]:
                        both_diag = (len(st["ents"]) == 2
                                     and st["ents"][0]["lo"] is not None
                                     and st["ents"][1]["lo"] is not None)
                        in1 = twin[:, 0:w] if both_diag else master[:, 512:512 + w]
                        nc.vector.scalar_tensor_tensor(
                            out=pt[:, lo:hi], in0=st["sp"][:, lo:hi],
                            scalar=0.0, in1=in1,
                            op0=mybir.AluOpType.max,
                            op1=mybir.AluOpType.mult)
                    elif st["eng"] == "dve":
                        nc.vector.tensor_scalar_max(
                            pt[:, lo:hi], st["sp"][:, lo:hi], 0.0)
                    else:
                        nc.scalar.activation(
                            pt[:, lo:hi], st["sp"][:, lo:hi],
                            mybir.ActivationFunctionType.Relu)

                def emit_pv(st):
                    if stage < 4:
                        return
                    pi = st["pair"]
                    pt = st["pt"]
                    for e in st["ents"]:
                        key = (pi, e["half"])
                        if key not in ytp_of:
                            ytp_of[key] = psmix.tile(
                                [64, TB], F32, tag="m",
                                name=f"ytp{pi}_{e['half']}")
                        ytp = ytp_of[key]
                        lo = e["lo"] if e["lo"] is not None else 0
                        cb = e["cb"]
                        kb = e["kb"]
                        voff = (kb % 2) * TB + e["head"] * 64
                        nc.tensor.matmul(
                            ytp[:, lo:TB],
                            v2_sb[kb // 2][:, voff:voff + 64],
                            pt[:, cb + lo:cb + TB],
                            start=e["start"], stop=e["stop"])
                    if st["last"]:
                        a, bb_ = pairs[pi]
                        for half, (h, qb) in enumerate((a, bb_)):
                            ytp = ytp_of[(pi, half)]
                            ys = ysp.tile([64, TB], F32, tag="ys",
                                          name=f"ys{pi}_{half}")
                            if st["ys_eng"][half] == "act":
                                nc.scalar.activation(
                                    ys, ytp,
                                    mybir.ActivationFunctionType.Identity)
                            else:
                                nc.vector.tensor_copy(ys, ytp)
                            nc.sync.dma_start(
                                out=yt_out[h, :, qb * TB:(qb + 1) * TB],
                                in_=ys)

                # ---- pipelined emission: SC(i+3), RL(i+2), PV(i) ----
                N = len(steps)
                emit_sc(steps[0])
                emit_sc(steps[1])
                emit_rl(steps[0])
                emit_sc(steps[2])
                emit_rl(steps[1])
                for i in range(N):
                    if i + 3 < N:
                        emit_sc(steps[i + 3])
                    if i + 2 < N:
                        emit_rl(steps[i + 2])
                    emit_pv(steps[i])

            if reps == 1:
                body()
            elif reps < 0:
                with tc.For_i(0, -reps, 1):
                    body()
            else:
                for _ in range(reps):
                    body()

    nc.finalize()
    return nc


def _prepare_in_maps(x, W_attn, b_attn):
    x = np.asarray(x, dtype=np.float32)
    W = np.asarray(W_attn, dtype=np.float32)
    bb = np.asarray(b_attn, dtype=np.float32)
    SC = np.float32(1.0 / np.sqrt(D))

    xT16 = [np.ascontiguousarray(x[b].T).astype(np.float16) for b in range(B)]

    in_maps = []
    for core in range(NCORES):
        b, g = divmod(core, NCORES // B)
        H = [g * HPC + h for h in range(HPC)]
        q_rows = [W[h * D:(h + 1) * D] for h in H]
        k_rows = [W[C + h * D:C + (h + 1) * D] for h in H]
        v_rows = [W[2 * C + h * D:2 * C + (h + 1) * D] for h in H]
        bq = [bb[h * D:(h + 1) * D] for h in H]
        bk = [bb[C + h * D:C + (h + 1) * D] for h in H]
        bv = [bb[2 * C + h * D:2 * C + (h + 1) * D] for h in H]

        # f-tiles: 0 = [q0; q1], 1 = [k0; k1], 2 = [k2; q2]
        wqk_rows = np.concatenate(
            [q_rows[0], q_rows[1], k_rows[0], k_rows[1], k_rows[2], q_rows[2]], 0)
        wqk16 = np.ascontiguousarray(wqk_rows.T).astype(np.float16)   # [768, 384]
        wv16 = np.ascontiguousarray(
            np.concatenate(v_rows, 0).T).astype(np.float16)           # [768, 192]

        bias_qk = np.stack([
            np.concatenate([bq[0], bq[1]]) * SC,
            np.concatenate([bk[0], bk[1]]),
            np.concatenate([bk[2], bq[2] * SC]),
        ]).astype(np.float32)                                          # [3, 128]
        scale_qk = np.stack([
            np.full(128, SC), np.ones(128),
            np.concatenate([np.ones(64), np.full(64, SC)]),
        ]).astype(np.float32)
        bv192 = np.concatenate(bv)                                     # [192]
        bv_row = np.zeros(1024, dtype=np.float32)
        for blk in range(2):
            bv_row[blk * 512:blk * 512 + 192] = bv192
        bias_v = np.tile(bv_row, (128, 1)).astype(np.float32)          # [128,1024]

        in_maps.append({
            "xT": xT16[b], "wqk": wqk16, "wv": wv16,
            "bias_qk": bias_qk, "scale_qk": scale_qk, "bias_v": bias_v,
        })
    return in_maps


_NC_CACHE = {}


def _get_nc(reps=1, stage=4):
    key = (reps, stage)
    if key not in _NC_CACHE:
        _NC_CACHE[key] = _build(reps, stage)
    return _NC_CACHE[key]


def kernel(x, W_attn, b_attn):
    nc = _get_nc(1)
    in_maps = _prepare_in_maps(x, W_attn, b_attn)
    res = run_bass_kernel_spmd(nc, in_maps, list(range(NCORES)), trace=False)
    y = np.empty((B, T, C), dtype=np.float32)
    for core in range(NCORES):
        b, g = divmod(core, NCORES // B)
        yt = res.results[core]["yt"]          # [3, 64, 2048]
        for h in range(HPC):
            y[b, :, (g * HPC + h) * D:(g * HPC + h + 1) * D] = yt[h].T
    return y


# revision 31
# speedup vs baseline: 1.1904x; 1.1904x over previous
"""Trainium2 Bass kernel for causal ReLU attention (no softmax).

  qkv = x @ W.T + b;  per head: s = (q k^T) * 1/sqrt(64)
  p = relu(causal(s));  y = p @ v

Sharding: 8 cores = 2 batches x 4 head-groups (3 heads each).

Per-core structure (v2):
  - qk-projection computed transposed (features on partitions) so q/k land
    as qT/kT [64, T]; scores use K=128 zero-padded k tiles (K<128 disables
    FWL and is ~4x slower per measurement; padding is free).
  - v-projection natural [T, 192]; pv matmuls are plain M=64 K=128 into
    per-chain [64, 512] PSUM tiles (tile_position pairing measured slower).
  - attention runs as 6 chain-pairs stepped in lockstep; relu+causal-mask
    is one fused op per step covering both chains' [128, 512] score
    blocks; diagonal masking via slices of master/twin triangle tiles
    (DVE scalar_tensor_tensor).  Engine per step chosen by greedy load
    balance (masked steps must use DVE).
  - global software pipeline: SC(i+3), RL(i+2), PV(i) with 3 score
    PSUM tiles (6 banks) + ytp/proj pool (2 banks) = 8 banks.
  - all DMA on the sync ring (scalar-ring DMA measured ~9x slower).
All matmul operands fp16 (fp32 PSUM accumulation). Host does the
shard/transpose/cast prep and the final gather (pure numpy).
"""
import numpy as np

import concourse.bass as bass
import concourse.mybir as mybir
import concourse.tile as tile
from concourse import bacc
from concourse.bass_utils import run_bass_kernel_spmd

F32 = mybir.dt.float32
F16 = mybir.dt.float16

B, T, C = 2, 2048, 768
NH = 12          # total heads
HPC = 3          # heads per core
D = 64
NCORES = 8
CC = 6           # contraction chunks (768 / 128)
TB = 512         # query block
KB = 128         # key block
NTB = T // TB    # 4
NKB = T // KB    # 16
MW = 1664        # master mask width

# cost model (ns) for engine balancing (ACT has measured 1.16x derate)
def _dve_cost(w):
    return (151.0 + w) / 0.96

def _act_cost(w):
    return 1.16 * (352.0 + w) / 1.2


def _build(reps=1, stage=4):
    nc = bacc.Bacc(None, target_bir_lowering=False, debug=False)
    xT = nc.declare_dram_parameter("xT", [C, T], F16, isOutput=False)
    wqk = nc.declare_dram_parameter("wqk", [C, 384], F16, isOutput=False)
    wv = nc.declare_dram_parameter("wv", [C, 192], F16, isOutput=False)
    bias_qk = nc.declare_dram_parameter("bias_qk", [3, 128], F32, isOutput=False)
    scale_qk = nc.declare_dram_parameter("scale_qk", [3, 128], F32, isOutput=False)
    bias_v = nc.declare_dram_parameter("bias_v", [128, 1024], F32, isOutput=False)
    yt_out = nc.declare_dram_parameter("yt", [HPC, D, T], F32, isOutput=True)

    with tile.TileContext(nc) as tc:
        with tc.tile_pool(name="const", bufs=1) as const, \
             tc.tile_pool(name="xr", bufs=12) as xr, \
             tc.tile_pool(name="vt", bufs=16) as vtp, \
             tc.tile_pool(name="pt", bufs=6) as ptp, \
             tc.tile_pool(name="ys", bufs=4) as ysp, \
             tc.tile_pool(name="psmix", bufs=2, space="PSUM") as psmix, \
             tc.tile_pool(name="pssc", bufs=3, space="PSUM") as pssc:

            # ---------------- constants ----------------
            bias_sb = const.tile([128, 3], F32)
            scale_sb = const.tile([128, 3], F32)
            nc.sync.dma_start(out=bias_sb, in_=bias_qk[:, :].rearrange("a p -> p a"))
            nc.sync.dma_start(out=scale_sb, in_=scale_qk[:, :].rearrange("a p -> p a"))
            biasv_sb = const.tile([128, 1024], F32)
            nc.sync.dma_start(out=biasv_sb, in_=bias_v[:, :])
            # master triangle: master[kk, m] = 1 if m >= 512 + kk else 0
            #   masked slice  : master[:, 512 : 512+w]  (diag at op-local col 0)
            #   all-ones slice: master[:, 640 : 640+w]
            master = const.tile([128, MW], F32)
            nc.vector.memset(master, 1.0)
            nc.gpsimd.affine_select(
                out=master, in_=master,
                compare_op=mybir.AluOpType.is_ge, fill=0.0, base=-512,
                pattern=[[1, MW]], channel_multiplier=-1)
            # twin triangles: twin[kk, m] = 1 if (m % 512) >= kk (for steps
            # where BOTH chains are diagonal blocks at the same shift)
            twin = const.tile([128, 2 * TB], F32)
            nc.vector.memset(twin, 1.0)
            for hh in range(2):
                nc.gpsimd.affine_select(
                    out=twin[:, hh * TB:(hh + 1) * TB],
                    in_=twin[:, hh * TB:(hh + 1) * TB],
                    compare_op=mybir.AluOpType.is_ge, fill=0.0, base=0,
                    pattern=[[1, TB]], channel_multiplier=-1)

            # weight chunks
            wqk_sb = [const.tile([128, 384], F16, tag="wqk", bufs=CC, name=f"wqk{c}") for c in range(CC)]
            wv_sb = [const.tile([128, 192], F16, tag="wv", bufs=CC, name=f"wv{c}") for c in range(CC)]
            for c in range(CC):
                nc.sync.dma_start(out=wqk_sb[c], in_=wqk[c * 128:(c + 1) * 128, :])
                nc.sync.dma_start(out=wv_sb[c], in_=wv[c * 128:(c + 1) * 128, :])

            # persistent attention operand tiles (zero-padded k selects the
            # head: contraction over 128 partitions hits zeros for the other)
            qq01 = const.tile([128, T], F16)   # [qT_h0; qT_h1] (scaled)
            kzA = const.tile([128, T], F16)    # [kT_h0; 0]
            kzB = const.tile([128, T], F16)    # [0; kT_h1]
            qq2 = const.tile([128, T], F16)    # [qT_h2 (dma); qT_h2 (act)]
            kz2 = const.tile([128, T], F16)    # [kT_h2; 0]
            nc.vector.memset(kzA[64:128, :], 0.0)
            nc.vector.memset(kzB[0:64, :], 0.0)
            nc.vector.memset(kz2[64:128, :], 0.0)

            def body():
                # stage: 1=dma only, 2=+proj, 3=+scores/relu, 4=full
                # probe stages: 21=xt+qkproj only, 22=xt+vproj only
                # ---------------- load xT (fp16, pre-cast on host) ---------
                xt = [xr.tile([128, T], F16, tag="xt", name=f"xt{c}") for c in range(CC)]
                for c in range(CC):
                    nc.sync.dma_start(out=xt[c], in_=xT[c * 128:(c + 1) * 128, :])

                if stage < 2:
                    return
                # ---------------- qk projection (transposed) ---------------
                # f-tiles: 0 = [q0; q1], 1 = [k0; k1], 2 = [k2; q2]
                # Two query-blocks per [128,1024] PSUM tile; two tiles'
                # accumulation chains interleaved so PSUM drains overlap
                # the next chain's streaming (chained mms into one region
                # otherwise serialize at ~490ns vs 136ns independent).
                Copy = mybir.ActivationFunctionType.Identity
                proj_eng = [0]

                def qk_emit(tiles):
                    # tiles: list of (ft, tb) -> one single-bank [128,512]
                    # psum tile each, chains interleaved c-major (two
                    # matmuls paired into one 2-bank tile measure ~77ns/mm
                    # slower than independent single-bank tiles)
                    ps_of = []
                    for ft, tb in tiles:
                        ps = pssc.tile([128, TB], F32, tag="s",
                                       name=f"pj{ft}_{tb}")
                        ps_of.append(ps)
                    for c in range(CC):
                        for (ft, tb), ps in zip(tiles, ps_of):
                            nc.tensor.matmul(
                                ps,
                                wqk_sb[c][:, ft * 128:(ft + 1) * 128],
                                xt[c][:, tb * TB:(tb + 1) * TB],
                                start=(c == 0), stop=(c == CC - 1))
                    for (ft, tb), ps in zip(tiles, ps_of):
                        ts = slice(tb * TB, (tb + 1) * TB)
                        if ft == 0:
                            dsts = [(qq01, slice(0, 128), 0)]
                        elif ft == 1:
                            dsts = [(kzA, slice(0, 64), 1),
                                    (kzB, slice(64, 128), 1)]
                        else:
                            dsts = [(kz2, slice(0, 64), 2),
                                    (qq2, slice(64, 128), 2)]
                        for dst, psl, col in dsts:
                            if proj_eng[0] % 2 == 0:
                                nc.scalar.activation(
                                    dst[psl, ts], ps[psl, :], Copy,
                                    bias=bias_sb[psl, col:col + 1],
                                    scale=scale_sb[psl, col:col + 1])
                            else:
                                nc.vector.tensor_scalar(
                                    out=dst[psl, ts], in0=ps[psl, :],
                                    scalar1=scale_sb[psl, col:col + 1],
                                    scalar2=bias_sb[psl, col:col + 1],
                                    op0=mybir.AluOpType.mult,
                                    op1=mybir.AluOpType.add)
                            proj_eng[0] += 1

                # ---------------- v projection (natural layout) ------------
                # One accumulation group per PSUM BANK (start=True clears
                # the whole bank, so groups must not share one): 2 token-
                # blocks per [128,1024] tile at cols 0/512; tile PAIRS
                # interleave for 4 independent chains (drain hiding).
                v2_sb = []

                def v_emit(tp_):
                    pss = [pssc.tile([128, 2 * TB], F32, tag="s",
                                     name=f"pv{tp_}_{j}") for j in range(2)]
                    for c in range(CC):
                        for j in range(2):
                            for blk in range(2):
                                tt = tp_ * 4 + j * 2 + blk
                                nc.tensor.matmul(
                                    pss[j][:, blk * TB:blk * TB + 192],
                                    xt[c][:, tt * 128:(tt + 1) * 128],
                                    wv_sb[c],
                                    start=(c == 0), stop=(c == CC - 1))
                    for j in range(2):
                        vt = vtp.tile([128, 2 * TB], F16, tag="v")
                        for blk in range(2):
                            nc.vector.tensor_add(
                                vt[:, blk * TB:blk * TB + 192],
                                pss[j][:, blk * TB:blk * TB + 192],
                                biasv_sb[:, blk * TB:blk * TB + 192])
                        v2_sb.append(vt)

                def qq2_shift():
                    # shift qT_h2 to partitions 0-63 (SBUF->SBUF DMA)
                    nc.sync.dma_start(out=qq2[0:64, :], in_=qq2[64:128, :])

                if stage < 3 or stage in (21, 22):
                    # probe stages: sequential emission
                    if stage != 22:
                        qk_emit([(0, 0), (0, 1), (0, 2)])
                        qk_emit([(0, 3), (1, 0), (1, 1)])
                        qk_emit([(1, 2), (1, 3), (2, 0)])
                        qk_emit([(2, 1), (2, 2), (2, 3)])
                    if stage not in (21, 22):
                        qq2_shift()
                    if stage != 21:
                        for tp_ in range(4):
                            v_emit(tp_)
                    return

                # full pipeline: heads 0/1 qk + first v quarter up front;
                # the rest of proj is woven into the attention stream at
                # pair boundaries so DVE/ACT relu work starts early.
                qk_emit([(0, 0), (0, 1), (0, 2)])
                qk_emit([(0, 3), (1, 0), (1, 1)])
                qk_emit([(1, 2), (1, 3)])
                v_emit(0)
                # ---------------- attention ----------------
                # chain operand lookup: head -> (q tile, zero-padded k tile)
                qk_of = {0: (qq01, kzA), 1: (qq01, kzB), 2: (qq2, kz2)}

                # pairs of (head, qb); first chain on partition half 0,
                # second on half 1.
                pairs = [((0, 0), (1, 0)), ((0, 1), (1, 1)),
                         ((0, 2), (1, 2)), ((0, 3), (1, 3)),
                         ((2, 0), (2, 1)), ((2, 2), (2, 3))]

                # ---- build step descriptors ----
                # entry: dict(head, qb, half, kb, lo(None if full), cb,
                #             start, stop)
                steps = []
                for pi, (ca, cb_) in enumerate(pairs):
                    chains = []
                    for half, (h, qb) in enumerate((ca, cb_)):
                        chains.append(dict(head=h, qb=qb, half=half,
                                           nkb=4 * qb + 4))
                    S = max(c["nkb"] for c in chains)
                    for t in range(S):
                        ents = []
                        for cch in chains:
                            if t >= cch["nkb"]:
                                continue
                            nfull = cch["nkb"] - 4
                            lo = None if t < nfull else (t - nfull) * KB
                            ents.append(dict(head=cch["head"], qb=cch["qb"],
                                             half=cch["half"], kb=t, lo=lo,
                                             start=(t == 0),
                                             stop=(t == cch["nkb"] - 1)))
                        # diag chain (if exactly one) goes in sp cols 0-511
                        if len(ents) == 2 and ents[0]["lo"] is None \
                                and ents[1]["lo"] is not None:
                            ents = [ents[1], ents[0]]
                        for j, e in enumerate(ents):
                            e["cb"] = j * TB
                        masked = any(e["lo"] is not None for e in ents)
                        op_lo = ents[0]["lo"] if ents[0]["lo"] is not None else 0
                        op_hi = TB if len(ents) == 1 else 2 * TB
                        steps.append(dict(pair=pi, t=t, ents=ents,
                                          masked=masked, op_lo=op_lo,
                                          op_hi=op_hi,
                                          last=(t == S - 1)))

                # ---- engine assignment (greedy balance) ----
                eng_t = {"dve": 0.0, "act": 0.0}
                for st in steps:
                    w = st["op_hi"] - st["op_lo"]
                    if st["masked"]:
                        st["eng"] = "dve"
                        eng_t["dve"] += _dve_cost(w)
                    else:
                        if eng_t["act"] + _act_cost(w) <= eng_t["dve"] + _dve_cost(w):
                            st["eng"] = "act"
                            eng_t["act"] += _act_cost(w)
                        else:
                            st["eng"] = "dve"
                            eng_t["dve"] += _dve_cost(w)
                    if st["last"]:  # two [64,512] ys copies follow
                        st["ys_eng"] = []
                        for _ in range(2):
                            if eng_t["act"] <= eng_t["dve"]:
                                st["ys_eng"].append("act")
                                eng_t["act"] += _act_cost(TB)
                            else:
                                st["ys_eng"].append("dve")
                                eng_t["dve"] += _dve_cost(TB)

                # ---- emission helpers ----
                ytp_of = {}

                def emit_sc(st):
                    sp = pssc.tile([128, 2 * TB], F32, tag="s",
                                   name=f"sp{st['pair']}_{st['t']}")
                    st["sp"] = sp
                    for e in st["ents"]:
                        qsb, ksb = qk_of[e["head"]]
                        qb, kb, cb = e["qb"], e["kb"], e["cb"]
                        lo = e["lo"]
                        # both-diag steps: second (full-slot) chain emitted
                        # unrestricted; garbage cols masked-out / unread.
                        if lo is None or (cb == TB):
                            q_ap = qsb[:, qb * TB:(qb + 1) * TB]
                            out_ap = sp[:, cb:cb + TB]
                        else:
                            q_ap = qsb[:, qb * TB + lo:(qb + 1) * TB]
                            out_ap = sp[:, cb + lo:cb + TB]
                        nc.tensor.matmul(
                            out_ap, ksb[:, kb * KB:(kb + 1) * KB], q_ap,
                            start=True, stop=True)

                def emit_rl(st):
                    pt = ptp.tile([128, 2 * TB], F16, tag="p2",
                                  name=f"pt{st['pair']}_{st['t']}")
                    st["pt"] = pt
                    lo, hi = st["op_lo"], st["op_hi"]
                    w = hi - lo
                    if st["masked"]:
                        both_diag = (len(st["ents"]) == 2
                                     and st["ents"][0]["lo"] is not None
                                     and st["ents"][1]["lo"] is not None)
                        in1 = twin[:, 0:w] if both_diag else master[:, 512:512 + w]
                        nc.vector.scalar_tensor_tensor(
                            out=pt[:, lo:hi], in0=st["sp"][:, lo:hi],
                            scalar=0.0, in1=in1,
                            op0=mybir.AluOpType.max,
                            op1=mybir.AluOpType.mult)
                    elif st["eng"] == "dve":
                        nc.vector.tensor_scalar_max(
                            pt[:, lo:hi], st["sp"][:, lo:hi], 0.0)
                    else:
                        nc.scalar.activation(
                            pt[:, lo:hi], st["sp"][:, lo:hi],
                            mybir.ActivationFunctionType.Relu)

                def emit_pv(st):
                    if stage < 4:
                        return
                    pi = st["pair"]
                    pt = st["pt"]
                    for e in st["ents"]:
                        key = (pi, e["half"])
                        if key not in ytp_of:
                            ytp_of[key] = psmix.tile(
                                [64, TB], F32, tag="m",
                                name=f"ytp{pi}_{e['half']}")
                        ytp = ytp_of[key]
                        lo = e["lo"] if e["lo"] is not None else 0
                        cb = e["cb"]
                        kb = e["kb"]
                        voff = (kb % 2) * TB + e["head"] * 64
                        nc.tensor.matmul(
                            ytp[:, lo:TB],
                            v2_sb[kb // 2][:, voff:voff + 64],
                            pt[:, cb + lo:cb + TB],
                            start=e["start"], stop=e["stop"])
                    if st["last"]:
                        a, bb_ = pairs[pi]
                        for half, (h, qb) in enumerate((a, bb_)):
                            ytp = ytp_of[(pi, half)]
                            ys = ysp.tile([64, TB], F32, tag="ys",
                                          name=f"ys{pi}_{half}")
                            if st["ys_eng"][half] == "act":
                                nc.scalar.activation(
                                    ys, ytp,
                                    mybir.ActivationFunctionType.Identity)
                            else:
                                nc.vector.tensor_copy(ys, ytp)
                            nc.sync.dma_start(
                                out=yt_out[h, :, qb * TB:(qb + 1) * TB],
                                in_=ys)

                # ---- pipelined emission: SC(i+3), RL(i+2), PV(i) ----
                # proj hooks fire right before the first SC of their pair
                first_idx = {}
                for idx, st in enumerate(steps):
                    first_idx.setdefault(st["pair"], idx)
                idx_hooks = {}

                def add_hook(pair, fn):
                    idx_hooks.setdefault(first_idx[pair], []).append(fn)

                add_hook(1, lambda: qk_emit([(2, 0), (2, 1)]))
                add_hook(1, lambda: qk_emit([(2, 2), (2, 3)]))
                add_hook(1, qq2_shift)
                add_hook(1, lambda: v_emit(1))
                add_hook(2, lambda: v_emit(2))
                add_hook(3, lambda: v_emit(3))

                def emit_sc_h(j):
                    for fn in idx_hooks.pop(j, []):
                        fn()
                    emit_sc(steps[j])

                N = len(steps)
                if stage == 24:  # probe: proj + scores only
                    for i in range(N):
                        emit_sc_h(i)
                    return
                emit_sc_h(0)
                emit_sc_h(1)
                emit_rl(steps[0])
                emit_sc_h(2)
                emit_rl(steps[1])
                for i in range(N):
                    if i + 3 < N:
                        emit_sc_h(i + 3)
                    if i + 2 < N:
                        emit_rl(steps[i + 2])
                    emit_pv(steps[i])

            if reps == 1:
                body()
            elif reps < 0:
                with tc.For_i(0, -reps, 1):
                    body()
            else:
                for _ in range(reps):
                    body()

    nc.finalize()
    return nc


def _prepare_in_maps(x, W_attn, b_attn):
    x = np.asarray(x, dtype=np.float32)
    W = np.asarray(W_attn, dtype=np.float32)
    bb = np.asarray(b_attn, dtype=np.float32)
    SC = np.float32(1.0 / np.sqrt(D))

    xT16 = [np.ascontiguousarray(x[b].T).astype(np.float16) for b in range(B)]

    in_maps = []
    for core in range(NCORES):
        b, g = divmod(core, NCORES // B)
        H = [g * HPC + h for h in range(HPC)]
        q_rows = [W[h * D:(h + 1) * D] for h in H]
        k_rows = [W[C + h * D:C + (h + 1) * D] for h in H]
        v_rows = [W[2 * C + h * D:2 * C + (h + 1) * D] for h in H]
        bq = [bb[h * D:(h + 1) * D] for h in H]
        bk = [bb[C + h * D:C + (h + 1) * D] for h in H]
        bv = [bb[2 * C + h * D:2 * C + (h + 1) * D] for h in H]

        # f-tiles: 0 = [q0; q1], 1 = [k0; k1], 2 = [k2; q2]
        wqk_rows = np.concatenate(
            [q_rows[0], q_rows[1], k_rows[0], k_rows[1], k_rows[2], q_rows[2]], 0)
        wqk16 = np.ascontiguousarray(wqk_rows.T).astype(np.float16)   # [768, 384]
        wv16 = np.ascontiguousarray(
            np.concatenate(v_rows, 0).T).astype(np.float16)           # [768, 192]

        bias_qk = np.stack([
            np.concatenate([bq[0], bq[1]]) * SC,
            np.concatenate([bk[0], bk[1]]),
            np.concatenate([bk[2], bq[2] * SC]),
        ]).astype(np.float32)                                          # [3, 128]
        scale_qk = np.stack([
            np.full(128, SC), np.ones(128),
            np.concatenate([np.ones(64), np.full(64, SC)]),
        ]).astype(np.float32)
        bv192 = np.concatenate(bv)                                     # [192]
        bv_row = np.zeros(1024, dtype=np.float32)
        for blk in range(2):
            bv_row[blk * 512:blk * 512 + 192] = bv192
        bias_v = np.tile(bv_row, (128, 1)).astype(np.float32)          # [128,1024]

        in_maps.append({
            "xT": xT16[b], "wqk": wqk16, "wv": wv16,
            "bias_qk": bias_qk, "scale_qk": scale_qk, "bias_v": bias_v,
        })
    return in_maps


_NC_CACHE = {}


def _get_nc(reps=1, stage=4):
    key = (reps, stage)
    if key not in _NC_CACHE:
        _NC_CACHE[key] = _build(reps, stage)
    return _NC_CACHE[key]


def kernel(x, W_attn, b_attn):
    nc = _get_nc(1)
    in_maps = _prepare_in_maps(x, W_attn, b_attn)
    res = run_bass_kernel_spmd(nc, in_maps, list(range(NCORES)), trace=False)
    y = np.empty((B, T, C), dtype=np.float32)
    for core in range(NCORES):
        b, g = divmod(core, NCORES // B)
        yt = res.results[core]["yt"]          # [3, 64, 2048]
        for h in range(HPC):
            y[b, :, (g * HPC + h) * D:(g * HPC + h + 1) * D] = yt[h].T
    return y
